# revision 4
# baseline (speedup 1.0000x reference)
"""MoE layer (dense experts) on 8 Trainium2 NeuronCores via Bass/Tile.

Problem (hardcoded shapes):
  x        [4, 2048, 1024] f32
  gate_w   [1024, 8] f32, gate_b [8] f32
  expert_w [8, 1024, 1024] f32, expert_b [8, 1024] f32
  out[b,t,p] = sum_e softmax(x @ gate_w + gate_b)[b,t,e] * (x @ expert_w[e] + expert_b[e])[b,t,p]

Sharding: data-parallel over tokens. 8192 tokens are split into 8 shards of
1024; every core gets the full gate/expert weights (replicated) and computes
its token shard end-to-end. No collectives.

Per-core kernel: x is pre-transposed on the host (xT [D, 1024]) so the
contraction dim is the SBUF partition dim for both matmul operands.
  - gating logits per token tile via PE matmuls accumulated over d-tiles,
    gate_b broadcast in via a K=1 rank-1 matmul (ones x gate_b row)
  - softmax on DVE/ACT (reduce_max -> Exp with accum_out -> reciprocal -> mul)
  - expert e: psum[t128, p512] = ones.T@expert_b[e] (K=1, start=True)
    + sum_d xT[d,t].T @ w_e[d,p]; accumulate over d in PSUM (9 matmuls/bank)
  - gate-weighted sum: acc = psum_e * g[:,e] + acc (one fused DVE
    scalar_tensor_tensor per psum tile)
Matmul dtype is float32r (full-rate fp32 streaming) by default; override
with MOE_MM_DTYPE in {fp32r, bf16, fp32}.
"""

import os
from contextlib import ExitStack

import numpy as np

import concourse.bacc as bacc
import concourse.bass as bass
import concourse.mybir as mybir
import concourse.tile as tile
from concourse.bass_utils import run_bass_kernel_spmd

B, T, D, E, P = 4, 2048, 1024, 8, 1024
N_CORES = 8
TOK = B * T                # 8192 tokens
TS = TOK // N_CORES        # 1024 tokens per core
DT = D // 128              # 8 contraction tiles
TT = TS // 128             # 8 token tiles per core
PCHUNK = 512               # psum bank free size (f32)
PC = P // PCHUNK           # 2 p-chunks

_F32 = mybir.dt.float32

MM_DTYPE = os.environ.get("MOE_MM_DTYPE", "fp32r")
TRACE = os.environ.get("MOE_TRACE", "0") == "1"

_mm_dt = {
    "fp32r": mybir.dt.float32r,
    "bf16": mybir.dt.bfloat16,
    "fp32": mybir.dt.float32,
}

_build_cache = {}


def _build(mode: str) -> bass.Bass:
    mm = _mm_dt[mode]
    nc = bacc.Bacc("TRN2", target_bir_lowering=False, debug=False,
                   num_devices=N_CORES)

    xT = nc.dram_tensor("xT", [D, TS], mm, kind="ExternalInput").ap()
    gw = nc.dram_tensor("gate_w", [D, E], mm, kind="ExternalInput").ap()
    gb = nc.dram_tensor("gate_b", [1, E], mm, kind="ExternalInput").ap()
    ew = nc.dram_tensor("expert_w", [E, D, P], mm, kind="ExternalInput").ap()
    eb = nc.dram_tensor("expert_b", [E, P], mm, kind="ExternalInput").ap()
    ones = nc.dram_tensor("ones", [1, 128], mm, kind="ExternalInput").ap()
    out = nc.dram_tensor("out", [TS, P], _F32, kind="ExternalOutput").ap()

    out_t = out.rearrange("(tt tp) p -> tp tt p", tp=128)

    with tile.TileContext(nc) as tc, ExitStack() as ctx:
        consts = ctx.enter_context(tc.tile_pool(name="consts", bufs=1))
        w_pool = ctx.enter_context(tc.tile_pool(name="w", bufs=2))
        acc_pool = ctx.enter_context(tc.tile_pool(name="acc", bufs=1))
        stats = ctx.enter_context(tc.tile_pool(name="stats", bufs=4))
        psum = ctx.enter_context(tc.tile_pool(name="psum", bufs=8, space="PSUM"))

        # Resident inputs
        xt = consts.tile([128, DT, TS], mm, name="xt")
        nc.sync.dma_start(xt[:, :, :], xT.rearrange("(dt dp) t -> dp dt t", dp=128))
        ones_sb = consts.tile([1, 128], mm, name="ones_sb")
        nc.sync.dma_start(ones_sb[:, :], ones)
        gw_sb = consts.tile([128, DT, E], mm, name="gw_sb")
        nc.sync.dma_start(gw_sb[:, :, :], gw.rearrange("(dt dp) e -> dp dt e", dp=128))
        gb_sb = consts.tile([1, E], mm, name="gb_sb")
        nc.sync.dma_start(gb_sb[:, :], gb)
        eb_sb = consts.tile([1, E, P], mm, name="eb_sb")
        nc.sync.dma_start(eb_sb[:, :, :], eb[None])

        g_sb = consts.tile([128, TT, E], _F32, name="g_sb")
        acc = acc_pool.tile([128, TT, P], _F32, name="acc")

        # --- gating: softmax(x @ gate_w + gate_b) per token tile ---
        for ti in range(TT):
            ps_g = psum.tile([128, PCHUNK], _F32, name="ps_g", tag="ps")
            lg = ps_g[:, :E]
            nc.tensor.matmul(lg, ones_sb[:1, :], gb_sb[:1, :],
                             start=True, stop=False)
            for di in range(DT):
                nc.tensor.matmul(lg, xt[:, di, ti * 128:(ti + 1) * 128],
                                 gw_sb[:, di, :],
                                 start=False, stop=(di == DT - 1))
            negmax = stats.tile([128, 1], _F32, name="negmax")
            nc.vector.tensor_reduce(negmax[:, :], lg, axis=mybir.AxisListType.X,
                                    op=mybir.AluOpType.max, negate=True)
            gexp = g_sb[:, ti, :]
            esum = stats.tile([128, 1], _F32, name="esum")
            nc.scalar.activation(gexp, lg, mybir.ActivationFunctionType.Exp,
                                 bias=negmax[:, :], scale=1.0,
                                 accum_out=esum[:, :])
            rec = stats.tile([128, 1], _F32, name="rec")
            nc.vector.reciprocal(rec[:, :], esum[:, :])
            nc.vector.tensor_scalar_mul(gexp, gexp, rec[:, :])

        # --- experts: psum = expert_b[e] + x @ w_e, acc += g[:,e] * psum ---
        for e in range(E):
            wt = w_pool.tile([128, DT, P], mm, name="wt", tag="wt")
            nc.sync.dma_start(wt[:, :, :],
                              ew[e].rearrange("(dt dp) p -> dp dt p", dp=128))
            for ti in range(TT):
                for pc in range(PC):
                    ps = psum.tile([128, PCHUNK], _F32, name="ps", tag="ps")
                    nc.tensor.matmul(
                        ps[:, :], ones_sb[:1, :],
                        eb_sb[:1, e, pc * PCHUNK:(pc + 1) * PCHUNK],
                        start=True, stop=False)
                    for di in range(DT):
                        nc.tensor.matmul(
                            ps[:, :], xt[:, di, ti * 128:(ti + 1) * 128],
                            wt[:, di, pc * PCHUNK:(pc + 1) * PCHUNK],
                            start=False, stop=(di == DT - 1))
                    g_col = g_sb[:, ti, e:e + 1]
                    acc_sl = acc[:, ti, pc * PCHUNK:(pc + 1) * PCHUNK]
                    if e == 0:
                        nc.vector.tensor_scalar_mul(acc_sl, ps[:, :], g_col)
                    else:
                        nc.vector.scalar_tensor_tensor(
                            acc_sl, ps[:, :], g_col, acc_sl,
                            op0=mybir.AluOpType.mult, op1=mybir.AluOpType.add)
                    if e == E - 1:
                        nc.sync.dma_start(
                            out_t[:, ti, pc * PCHUNK:(pc + 1) * PCHUNK], acc_sl)

    nc.compile()
    return nc


def _get_module(mode: str) -> bass.Bass:
    if mode not in _build_cache:
        _build_cache[mode] = _build(mode)
    return _build_cache[mode]


_last_results = None


def kernel(x, gate_w, gate_b, expert_w, expert_b):
    global _last_results
    mode = MM_DTYPE
    nc = _get_module(mode)

    np_dt = np.float32
    if mode == "bf16":
        import ml_dtypes
        np_dt = ml_dtypes.bfloat16

    x_flat = np.asarray(x, dtype=np.float32).reshape(TOK, D)
    gate_w = np.asarray(gate_w, dtype=np.float32)
    gate_b = np.asarray(gate_b, dtype=np.float32).reshape(1, E)
    expert_w = np.asarray(expert_w, dtype=np.float32)
    expert_b = np.asarray(expert_b, dtype=np.float32)

    gw_h = np.ascontiguousarray(gate_w).astype(np_dt)
    gb_h = np.ascontiguousarray(gate_b).astype(np_dt)
    ew_h = np.ascontiguousarray(expert_w).astype(np_dt)
    eb_h = np.ascontiguousarray(expert_b).astype(np_dt)
    ones_h = np.ones((1, 128), dtype=np_dt)

    in_maps = []
    for c in range(N_CORES):
        shard = x_flat[c * TS:(c + 1) * TS]           # [TS, D]
        xT_h = np.ascontiguousarray(shard.T).astype(np_dt)  # [D, TS]
        in_maps.append({
            "xT": xT_h, "gate_w": gw_h, "gate_b": gb_h,
            "expert_w": ew_h, "expert_b": eb_h, "ones": ones_h,
        })

    res = run_bass_kernel_spmd(nc, in_maps, core_ids=list(range(N_CORES)),
                               trace=TRACE)
    _last_results = res

    out = np.concatenate([res.results[c]["out"] for c in range(N_CORES)], axis=0)
    return out.reshape(B, T, P).astype(np.float32)


# revision 5
# speedup vs baseline: 1.0992x; 1.0992x over previous
"""MoE layer (dense experts) on 8 Trainium2 NeuronCores via Bass/Tile.

Problem (hardcoded shapes):
  x        [4, 2048, 1024] f32
  gate_w   [1024, 8] f32, gate_b [8] f32
  expert_w [8, 1024, 1024] f32, expert_b [8, 1024] f32
  out[b,t,p] = sum_e softmax(x @ gate_w + gate_b)[b,t,e]
               * (x @ expert_w[e] + expert_b[e])[b,t,p]

Sharding: data-parallel over tokens. 8192 tokens are split into 8 shards of
1024; every core gets the full gate/expert weights (replicated) and computes
its token shard end-to-end. No collectives.

Per-core kernel (x pre-transposed on host so the contraction dim is the
partition dim for both matmul operands):
  - gating logits per token tile via PE matmuls accumulated over d-tiles
    (gate_b broadcast in via a K=1 ones x gate_b rank-1 matmul), softmax on
    DVE/ACT, normalized gates also transposed on PE for the bias-mix matmul
  - expert e: psum[t128, p512] accumulates sum_d xT[d,t].T @ w_e[d,p] over
    8 d-tiles; d is the outer loop within a 4-token-tile half so compute
    starts as soon as the first w d-tile DMA lands
  - gate-weighted sum on DVE: acc = psum_e * g[:,e] + acc (one fused
    scalar_tensor_tensor per psum tile)
  - expert_b handled once per output tile: psum_b = gT.T @ expert_b (K=8
    matmul, gate-weighted bias mix), final out = acc + psum_b
Matmul dtype: bf16 (default) or float32r (full-rate fp32 streaming, ~1.2x
slower, ~16x more accurate) via MOE_MM_DTYPE in {bf16, fp32r, fp32}.
"""

import os
from contextlib import ExitStack

import numpy as np

import concourse.bacc as bacc
import concourse.bass as bass
import concourse.mybir as mybir
import concourse.tile as tile
from concourse.bass_utils import run_bass_kernel_spmd

B, T, D, E, P = 4, 2048, 1024, 8, 1024
N_CORES = 8
TOK = B * T                # 8192 tokens
TS = TOK // N_CORES        # 1024 tokens per core
DT = D // 128              # 8 contraction tiles
TT = TS // 128             # 8 token tiles per core
PCHUNK = 512               # psum bank free size (f32)
PC = P // PCHUNK           # 2 p-chunks
TH = 4                     # token tiles per half (TH*PC = 8 psum banks)

_F32 = mybir.dt.float32
_BF16 = mybir.dt.bfloat16

MM_DTYPE = os.environ.get("MOE_MM_DTYPE", "bf16")
TRACE = os.environ.get("MOE_TRACE", "0") == "1"

_mm_dt = {
    "fp32r": mybir.dt.float32r,
    "bf16": mybir.dt.bfloat16,
    "fp32": mybir.dt.float32,
}

_build_cache = {}


def _build(mode: str) -> bass.Bass:
    mm = _mm_dt[mode]
    nc = bacc.Bacc("TRN2", target_bir_lowering=False, debug=False,
                   num_devices=N_CORES)

    xT = nc.dram_tensor("xT", [D, TS], mm, kind="ExternalInput").ap()
    gw = nc.dram_tensor("gate_w", [D, E], mm, kind="ExternalInput").ap()
    gb = nc.dram_tensor("gate_b", [1, E], mm, kind="ExternalInput").ap()
    ew = nc.dram_tensor("expert_w", [E, D, P], mm, kind="ExternalInput").ap()
    eb = nc.dram_tensor("expert_b", [E, P], _BF16, kind="ExternalInput").ap()
    ones = nc.dram_tensor("ones", [1, 128], mm, kind="ExternalInput").ap()
    ident = nc.dram_tensor("ident", [128, 128], _F32, kind="ExternalInput").ap()
    out = nc.dram_tensor("out", [TS, P], _F32, kind="ExternalOutput").ap()

    out_t = out.rearrange("(tt tp) p -> tp tt p", tp=128)
    xT_t = xT.rearrange("(dt dp) t -> dp dt t", dp=128)

    with tile.TileContext(nc) as tc, ExitStack() as ctx:
        consts = ctx.enter_context(tc.tile_pool(name="consts", bufs=1))
        w_pool = ctx.enter_context(tc.tile_pool(name="w", bufs=12))
        stage_pool = ctx.enter_context(tc.tile_pool(name="stage", bufs=4))
        stats = ctx.enter_context(tc.tile_pool(name="stats", bufs=4))
        psum = ctx.enter_context(tc.tile_pool(name="psum", bufs=8, space="PSUM"))

        # Resident inputs. xT is loaded per d-tile so consumers start early.
        xt = consts.tile([128, DT, TS], mm, name="xt")
        for di in range(DT):
            nc.sync.dma_start(xt[:, di, :], xT_t[:, di, :])
        ones_sb = consts.tile([1, 128], mm, name="ones_sb")
        nc.sync.dma_start(ones_sb[:, :], ones)
        gw_sb = consts.tile([128, DT, E], mm, name="gw_sb")
        nc.sync.dma_start(gw_sb[:, :, :], gw.rearrange("(dt dp) e -> dp dt e", dp=128))
        gb_sb = consts.tile([1, E], mm, name="gb_sb")
        nc.sync.dma_start(gb_sb[:, :], gb)
        eb_sb = consts.tile([E, P], _BF16, name="eb_sb")
        nc.sync.dma_start(eb_sb[:, :], eb)
        id_sb = consts.tile([128, 128], _F32, name="id_sb")
        nc.sync.dma_start(id_sb[:, :], ident)

        g_sb = consts.tile([128, TT, E], _F32, name="g_sb")
        gt_sb = consts.tile([E, TS], _BF16, name="gt_sb")
        acc = consts.tile([128, TT, P], _F32, name="acc")

        # --- gating: g = softmax(x @ gate_w + gate_b), plus gT for the
        # bias-mix matmul ---
        for ti in range(TT):
            ps_g = psum.tile([128, PCHUNK], _F32, name="ps_g", tag="ps")
            lg = ps_g[:, :E]
            nc.tensor.matmul(lg, ones_sb[:1, :], gb_sb[:1, :],
                             start=True, stop=False)
            for di in range(DT):
                nc.tensor.matmul(lg, xt[:, di, ti * 128:(ti + 1) * 128],
                                 gw_sb[:, di, :],
                                 start=False, stop=(di == DT - 1))
            negmax = stats.tile([128, 1], _F32, name="negmax")
            nc.vector.tensor_reduce(negmax[:, :], lg, axis=mybir.AxisListType.X,
                                    op=mybir.AluOpType.max, negate=True)
            gexp = g_sb[:, ti, :]
            esum = stats.tile([128, 1], _F32, name="esum")
            nc.scalar.activation(gexp, lg, mybir.ActivationFunctionType.Exp,
                                 bias=negmax[:, :], scale=1.0,
                                 accum_out=esum[:, :])
            rec = stats.tile([128, 1], _F32, name="rec")
            nc.vector.reciprocal(rec[:, :], esum[:, :])
            nc.vector.tensor_scalar_mul(gexp, gexp, rec[:, :])
            # gT[e, t] for the expert_b bias-mix matmul
            ps_t = psum.tile([128, PCHUNK], _F32, name="ps_t", tag="ps")
            gt_ps = ps_t[:E, :128]
            nc.tensor.transpose(gt_ps, gexp, id_sb[:, :])
            nc.scalar.copy(gt_sb[:, ti * 128:(ti + 1) * 128], gt_ps)

        # --- experts ---
        for e in range(E):
            wt = []
            for di in range(DT):
                w_tile = w_pool.tile([128, P], mm, name=f"wt{e}_{di}", tag="wt")
                nc.sync.dma_start(w_tile[:, :], ew[e, di * 128:(di + 1) * 128, :])
                wt.append(w_tile)
            for half in range(TT // TH):
                tis = range(half * TH, (half + 1) * TH)
                ps_grp = {}
                for ti in tis:
                    for pc in range(PC):
                        ps_grp[ti, pc] = psum.tile([128, PCHUNK], _F32,
                                                   name=f"ps{e}_{ti}_{pc}",
                                                   tag="ps")
                for di in range(DT):
                    for ti in tis:
                        for pc in range(PC):
                            nc.tensor.matmul(
                                ps_grp[ti, pc][:, :],
                                xt[:, di, ti * 128:(ti + 1) * 128],
                                wt[di][:, pc * PCHUNK:(pc + 1) * PCHUNK],
                                start=(di == 0), stop=(di == DT - 1))
                for ti in tis:
                    for pc in range(PC):
                        ps = ps_grp[ti, pc]
                        g_col = g_sb[:, ti, e:e + 1]
                        acc_sl = acc[:, ti, pc * PCHUNK:(pc + 1) * PCHUNK]
                        if e == 0:
                            nc.vector.tensor_scalar_mul(acc_sl, ps[:, :], g_col)
                        else:
                            nc.vector.scalar_tensor_tensor(
                                acc_sl, ps[:, :], g_col, acc_sl,
                                op0=mybir.AluOpType.mult,
                                op1=mybir.AluOpType.add)
                        if e == E - 1:
                            # gate-weighted expert_b mix + final store
                            ps_b = psum.tile([128, PCHUNK], _F32,
                                             name=f"psb{ti}_{pc}", tag="ps")
                            nc.tensor.matmul(
                                ps_b[:, :],
                                gt_sb[:, ti * 128:(ti + 1) * 128],
                                eb_sb[:, pc * PCHUNK:(pc + 1) * PCHUNK],
                                start=True, stop=True)
                            stg = stage_pool.tile([128, PCHUNK], _F32,
                                                  name="stg")
                            nc.vector.tensor_add(stg[:, :], acc_sl, ps_b[:, :])
                            nc.sync.dma_start(
                                out_t[:, ti, pc * PCHUNK:(pc + 1) * PCHUNK],
                                stg[:, :])

    nc.compile()
    return nc


def _get_module(mode: str) -> bass.Bass:
    if mode not in _build_cache:
        _build_cache[mode] = _build(mode)
    return _build_cache[mode]


_last_results = None


def _host_inputs(x, gate_w, gate_b, expert_w, expert_b, mode):
    import ml_dtypes
    np_dt = ml_dtypes.bfloat16 if mode == "bf16" else np.float32

    x_flat = np.asarray(x, dtype=np.float32).reshape(TOK, D)
    gw_h = np.ascontiguousarray(np.asarray(gate_w, np.float32)).astype(np_dt)
    gb_h = np.asarray(gate_b, np.float32).reshape(1, E).astype(np_dt)
    ew_h = np.ascontiguousarray(np.asarray(expert_w, np.float32)).astype(np_dt)
    eb_h = np.asarray(expert_b, np.float32).astype(ml_dtypes.bfloat16)
    ones_h = np.ones((1, 128), dtype=np_dt)
    ident_h = np.eye(128, dtype=np.float32)

    in_maps = []
    for c in range(N_CORES):
        shard = x_flat[c * TS:(c + 1) * TS]                  # [TS, D]
        xT_h = np.ascontiguousarray(shard.T).astype(np_dt)   # [D, TS]
        in_maps.append({
            "xT": xT_h, "gate_w": gw_h, "gate_b": gb_h,
            "expert_w": ew_h, "expert_b": eb_h, "ones": ones_h,
            "ident": ident_h,
        })
    return in_maps


def kernel(x, gate_w, gate_b, expert_w, expert_b):
    global _last_results
    mode = MM_DTYPE
    nc = _get_module(mode)
    in_maps = _host_inputs(x, gate_w, gate_b, expert_w, expert_b, mode)

    res = run_bass_kernel_spmd(nc, in_maps, core_ids=list(range(N_CORES)),
                               trace=TRACE)
    _last_results = res

    out = np.concatenate([res.results[c]["out"] for c in range(N_CORES)], axis=0)
    return out.reshape(B, T, P).astype(np.float32)
